# revision 14
# baseline (speedup 1.0000x reference)
"""Trainium2 Bass kernel for nn_AttentionBlock (GroupNorm + 4-head attention + proj + residual).

Sharding: 8 cores; core handles batch b = core//2 and head pair p = core%2
(global heads 2p, 2p+1). Each core computes GroupNorm(x_b), its heads' q/k/v,
full 4096x4096 attention for its 2 heads, and a partial projection output.
Host sums the two partial projections per batch element and adds the residual.

Layout strategy (channels-on-partitions throughout):
  xn   [c=128, T]      groupnormed input (bf16)
  Qrep [128, T]        Q^T replicated 2x per head: rows 32q hold head q//2 (bf16)
  Kq   [128, T/2]      K^T quadrant-packed: row 64*hl+32*(jc%2), col (jc//2)*128 (bf16)
  V'   [128, 65*JC]    per j-chunk [V_h0(32) | ones(1) | V_h1(32)] (bf16)
  S^T  psum [128,1024] 2-row-packed QK^T matmuls (K=32 contraction via tile_position)
  A^T  [128, 1024]     exp(scale*S^T) (ACT, bf16)
  O'   psum [33, 512]  (A@V)^T accumulated over j; extra ones-row = softmax denom r
  h    [64, T]         O'/r (bf16), both heads stacked
  part [128, T]        proj_w[:, 64p:64p+64] @ h  (fp32, DMA'd out)
"""

import numpy as np
import ml_dtypes

BF16 = ml_dtypes.bfloat16

_NC_CACHE = {}
_RUNNER_CACHE = {}


def _install_neff_cache():
    """Disk-cache walrus-compiled NEFFs keyed by BIR hash (speeds re-runs)."""
    import concourse.bass_utils as bu
    import concourse.bass2jax as b2j
    import hashlib, os, shutil

    if getattr(bu, "_neff_cache_installed", False):
        return
    orig = bu.compile_bir_kernel

    def cached(bir_json, tmpdir, neff_name="file.neff", **kw):
        data = bir_json if isinstance(bir_json, bytes) else bir_json.encode()
        h = hashlib.sha256(data).hexdigest()[:24]
        cdir = os.environ.get("BASS_NEFF_CACHE_DIR", "/tmp/bass_neff_cache")
        try:
            os.makedirs(cdir, exist_ok=True)
        except OSError:
            return orig(bir_json, tmpdir, neff_name=neff_name, **kw)
        cpath = os.path.join(cdir, h + ".neff")
        if os.path.exists(cpath):
            return cpath
        p = orig(bir_json, tmpdir, neff_name=neff_name, **kw)
        try:
            shutil.copy(p, cpath)
        except OSError:
            pass
        return p

    bu.compile_bir_kernel = cached
    b2j.compile_bir_kernel = cached
    bu._neff_cache_installed = True


def _build(T, with_vbias, repeats=1):
    import concourse.bacc as bacc
    import concourse.tile as tile
    from concourse import mybir

    f32 = mybir.dt.float32
    bf16 = mybir.dt.bfloat16
    IC = T // 512          # i chunks (512 wide)
    JC = T // 128          # j chunks (128 wide)
    JP = JC // 2           # packed j pairs
    KQCOL = (JC // 2) * 128  # = T/2
    NB = T // 512          # bn_stats chunks
    SCALE = 1.0 / np.sqrt(32.0)

    nc = bacc.Bacc("TRN2", target_bir_lowering=False, debug=False)

    x_d = nc.dram_tensor("x", [128, T], f32, kind="ExternalInput").ap()
    gnw_d = nc.dram_tensor("gnw", [128, 1], f32, kind="ExternalInput").ap()
    gnb_d = nc.dram_tensor("gnb", [128, 1], f32, kind="ExternalInput").ap()
    wq_d = nc.dram_tensor("wq_rep", [128, 128], bf16, kind="ExternalInput").ap()
    wk_d = nc.dram_tensor("wk_duo", [128, 64], bf16, kind="ExternalInput").ap()
    wv_d = nc.dram_tensor("wv_duo", [128, 64], bf16, kind="ExternalInput").ap()
    wp_d = nc.dram_tensor("wp_t", [64, 128], bf16, kind="ExternalInput").ap()
    qb_d = nc.dram_tensor("qb_rep", [128, 1], f32, kind="ExternalInput").ap()
    kb_d = nc.dram_tensor("kb_duo", [128, 1], f32, kind="ExternalInput").ap()
    vb_d = nc.dram_tensor("vb_duo", [64, 1], f32, kind="ExternalInput").ap()
    oh_d = nc.dram_tensor("oh", [128, 8], f32, kind="ExternalInput").ap()
    ohT_d = nc.dram_tensor("ohT", [8, 128], f32, kind="ExternalInput").ap()
    part_d = nc.dram_tensor("part", [128, T], f32, kind="ExternalOutput").ap()

    with tile.TileContext(nc) as tc:
      for _rep in range(repeats):
        # timing variant (repeats>1): chain rep N's input to rep N-1's output
        if _rep > 0:
            x_d = part_d.bitcast(f32)
        with (
            tc.tile_pool(name=f"consts{_rep}", bufs=1) as consts,
            tc.tile_pool(name=f"big{_rep}", bufs=1) as big,
            tc.tile_pool(name=f"at{_rep}", bufs=3) as atp,
            tc.tile_pool(name=f"tmp{_rep}", bufs=4) as tmp,
            tc.tile_pool(name=f"rb{_rep}", bufs=2) as rbp,
            tc.tile_pool(name=f"ps_c{_rep}", bufs=2, space="PSUM") as ps_c,
            tc.tile_pool(name=f"ps_st{_rep}", bufs=2, space="PSUM") as ps_st,
            tc.tile_pool(name=f"ps_o{_rep}", bufs=2, space="PSUM") as ps_o,
        ):
            # ---- constants / weights in ----
            gnw = consts.tile([128, 1], f32, tag="gnw")
            gnb = consts.tile([128, 1], f32, tag="gnb")
            wq = consts.tile([128, 128], bf16, tag="wq")
            wk = consts.tile([128, 64], bf16, tag="wk")
            wv = consts.tile([128, 64], bf16, tag="wv")
            wp = consts.tile([64, 128], bf16, tag="wp")
            qb = consts.tile([128, 1], f32, tag="qb")
            kb = consts.tile([128, 1], f32, tag="kb")
            vb = consts.tile([64, 1], f32, tag="vb")
            nc.sync.dma_start(out=gnw, in_=gnw_d)
            nc.sync.dma_start(out=gnb, in_=gnb_d)
            nc.sync.dma_start(out=wq, in_=wq_d)
            nc.sync.dma_start(out=wk, in_=wk_d)
            nc.sync.dma_start(out=wv, in_=wv_d)
            nc.sync.dma_start(out=wp, in_=wp_d)
            nc.sync.dma_start(out=qb, in_=qb_d)
            nc.sync.dma_start(out=kb, in_=kb_d)
            nc.sync.dma_start(out=vb, in_=vb_d)

            eps_t = consts.tile([128, 1], f32, tag="eps")
            nc.vector.memset(eps_t, 1e-5)
            ones32 = consts.tile([1, 32], bf16, tag="ones32")
            nc.vector.memset(ones32, 1.0)
            # group one-hot (value 1/16 folds in the group-average) and its transpose
            oh = consts.tile([128, 8], f32, tag="oh")
            nc.sync.dma_start(out=oh, in_=oh_d)
            ohT = consts.tile([8, 128], f32, tag="ohT")
            nc.sync.dma_start(out=ohT, in_=ohT_d)

            # ---- x in ----
            x_sb = big.tile([128, T], f32, tag="x")
            nc.sync.dma_start(out=x_sb, in_=x_d)

            # ---- GroupNorm stats ----
            stats = tmp.tile([128, NB, 6], f32, tag="bnst")
            for i in range(NB):
                nc.vector.bn_stats(out=stats[:, i, :], in_=x_sb[:, 512 * i : 512 * i + 512])
            mv = tmp.tile([128, 2], f32, tag="mv")
            nc.vector.bn_aggr(out=mv, in_=stats)
            # per-channel (mean, E[x^2])
            cs = tmp.tile([128, 2], f32, tag="cs")
            nc.vector.tensor_copy(cs[:, 0:1], mv[:, 0:1])
            msq = tmp.tile([128, 1], f32, tag="msq")
            nc.vector.tensor_mul(msq, mv[:, 0:1], mv[:, 0:1])
            nc.vector.tensor_add(cs[:, 1:2], mv[:, 1:2], msq)
            # group-average via one-hot matmul (fp32), then broadcast back
            gs_ps = ps_c.tile([8, 2], f32, tag="c")
            nc.tensor.matmul(gs_ps, oh, cs)
            gs_sb = tmp.tile([8, 2], f32, tag="gs")
            nc.vector.tensor_copy(gs_sb, gs_ps)
            cb_ps = ps_c.tile([128, 2], f32, tag="c")
            nc.tensor.matmul(cb_ps, ohT, gs_sb)
            cb = tmp.tile([128, 2], f32, tag="cb")
            nc.vector.tensor_copy(cb, cb_ps)
            # rstd = 1/sqrt(E[x^2] - mean^2 + eps)
            gmsq = tmp.tile([128, 1], f32, tag="gmsq")
            nc.vector.tensor_mul(gmsq, cb[:, 0:1], cb[:, 0:1])
            gvar = tmp.tile([128, 1], f32, tag="gvar")
            nc.vector.tensor_sub(gvar, cb[:, 1:2], gmsq)
            nc.scalar.activation(
                out=gvar, in_=gvar, func=mybir.ActivationFunctionType.Sqrt,
                bias=eps_t, scale=1.0,
            )
            rstd = tmp.tile([128, 1], f32, tag="rstd")
            nc.vector.reciprocal(out=rstd, in_=gvar)
            # affine: xn = x * (norm_w * rstd) + (norm_b - mean * norm_w * rstd)
            s_ap = tmp.tile([128, 1], f32, tag="s_ap")
            nc.vector.tensor_mul(s_ap, gnw, rstd)
            mb = tmp.tile([128, 1], f32, tag="mb")
            nc.vector.tensor_mul(mb, cb[:, 0:1], s_ap)
            b_ap = tmp.tile([128, 1], f32, tag="b_ap")
            nc.vector.tensor_sub(b_ap, gnb, mb)
            xn = big.tile([128, T], bf16, tag="xn")
            nc.vector.tensor_scalar(
                out=xn, in0=x_sb, scalar1=s_ap, scalar2=b_ap,
                op0=mybir.AluOpType.mult, op1=mybir.AluOpType.add,
            )

            # ---- QKV ----
            q_rep = big.tile([128, T], bf16, tag="q_rep")
            for i in range(IC):
                qp = ps_c.tile([128, 512], f32, tag="c")
                nc.tensor.matmul(qp, wq, xn[:, 512 * i : 512 * i + 512])
                nc.vector.tensor_scalar(
                    out=q_rep[:, 512 * i : 512 * i + 512], in0=qp,
                    scalar1=qb, scalar2=None, op0=mybir.AluOpType.add,
                )

            k_q = big.tile([128, KQCOL], bf16, tag="k_q")
            RW = min(512, KQCOL)
            for t in range((KQCOL + 511) // 512):
                kp = ps_c.tile([128, RW], f32, tag="c")
                for hl in range(2):
                    for jj in range(min(8, JC - 8 * t)):
                        jc = 8 * t + jj
                        nc.tensor.matmul(
                            kp[64 * hl + 32 * (jc % 2) : 64 * hl + 32 * (jc % 2) + 32,
                               ((jc // 2) % 4) * 128 : ((jc // 2) % 4) * 128 + 128],
                            wk[:, 32 * hl : 32 * hl + 32],
                            xn[:, 128 * jc : 128 * jc + 128],
                            tile_position=(0, 64 * hl + 32 * (jc % 2)),
                            skip_group_check=True,
                        )
                nc.vector.tensor_scalar(
                    out=k_q[:, 512 * t : 512 * t + RW], in0=kp,
                    scalar1=kb, scalar2=None, op0=mybir.AluOpType.add,
                )

            # per j-chunk block: [V_h0(0:32) | ones(32) | V_h1(33:65) | ones(65)]
            v_sb = big.tile([128, 66 * JC], bf16, tag="v_sb")
            nc.vector.memset(v_sb[:, 32 : 32 + 66 * (JC - 1) + 1 : 66], 1.0)
            nc.vector.memset(v_sb[:, 65 : 65 + 66 * (JC - 1) + 1 : 66], 1.0)
            for jc in range(JC):
                vp = ps_c.tile([128, 64], f32, tag="c")
                nc.tensor.matmul(vp, xn[:, 128 * jc : 128 * jc + 128], wv)
                nc.vector.tensor_copy(v_sb[:, 66 * jc : 66 * jc + 32], vp[:, 0:32])
                nc.vector.tensor_copy(v_sb[:, 66 * jc + 33 : 66 * jc + 65], vp[:, 32:64])

            # ---- attention ----
            h_sb = big.tile([64, T], bf16, tag="h_sb")
            for hl in range(2):
                for i in range(IC):
                    op = ps_o.tile([33, 512], f32, tag="o")
                    for jp in range(JP):
                        st = ps_st.tile([128, 1024], f32, tag="st")
                        for g in range(2):
                            jc = 2 * jp + g
                            nc.tensor.matmul(
                                st[:, 512 * g : 512 * g + 512],
                                k_q[64 * hl + 32 * (jc % 2) : 64 * hl + 32 * (jc % 2) + 32,
                                    ((jc // 2) % 4) * 128 + 512 * (jc // 8) : ((jc // 2) % 4) * 128 + 512 * (jc // 8) + 128],
                                q_rep[64 * hl + 32 * (jc % 2) : 64 * hl + 32 * (jc % 2) + 32,
                                      512 * i : 512 * i + 512],
                                tile_position=(64 * hl + 32 * (jc % 2), 0),
                                skip_group_check=True,
                            )
                        at = atp.tile([128, 1024], bf16, tag="at")
                        nc.scalar.activation(
                            out=at, in_=st, func=mybir.ActivationFunctionType.Exp,
                            scale=SCALE,
                        )
                        for g in range(2):
                            jc = 2 * jp + g
                            nc.tensor.matmul(
                                op,
                                v_sb[:, 66 * jc + 33 * hl : 66 * jc + 33 * hl + 33],
                                at[:, 512 * g : 512 * g + 512],
                                start=(jp == 0 and g == 0),
                                stop=(jp == JP - 1 and g == 1),
                                skip_group_check=True,
                            )
                    # normalize: h = O'[d] / r  (r is the ones-column row)
                    d_rows, r_row = op[0:32, :], op[32:33, :]
                    rinv = tmp.tile([1, 512], bf16, tag="rinv")
                    with nc.allow_low_precision(reason="softmax denom fits bf16"):
                        nc.vector.reciprocal(out=rinv, in_=r_row)
                    rb_ps = ps_c.tile([32, 512], f32, tag="c")
                    nc.tensor.matmul(rb_ps, ones32, rinv)
                    rb_sb = rbp.tile([32, 512], bf16, tag="rb")
                    nc.vector.tensor_copy(rb_sb, rb_ps)
                    h_slice = h_sb[32 * hl : 32 * hl + 32, 512 * i : 512 * i + 512]
                    nc.vector.tensor_mul(h_slice, d_rows, rb_sb)
                    if with_vbias:
                        nc.vector.tensor_scalar(
                            out=h_slice, in0=h_slice,
                            scalar1=vb[32 * hl : 32 * hl + 32, :], scalar2=None,
                            op0=mybir.AluOpType.add,
                        )

            # ---- proj ----
            out_sb = big.tile([128, T], f32, tag="out_sb")
            for i in range(IC):
                pp = ps_c.tile([128, 512], f32, tag="c")
                nc.tensor.matmul(pp, wp, h_sb[:, 512 * i : 512 * i + 512])
                nc.vector.tensor_copy(out_sb[:, 512 * i : 512 * i + 512], pp)
            nc.sync.dma_start(out=part_d, in_=out_sb)

    nc.compile()
    return nc


def _get_nc(T, with_vbias):
    key = (T, with_vbias)
    if key not in _NC_CACHE:
        _NC_CACHE[key] = _build(T, with_vbias)
    return _NC_CACHE[key]


def _make_in_maps(x, norm_w, norm_b, qkv_w, qkv_b, proj_w):
    b, c, hh, ww = x.shape
    T = hh * ww
    xf = np.ascontiguousarray(x.reshape(b, c, T), dtype=np.float32)
    in_maps = []
    for core in range(8):
        p = core % 2
        bi = core // 2
        h0, h1 = 2 * p, 2 * p + 1
        wqT0 = qkv_w[32 * h0 : 32 * h0 + 32, :].T  # [128, 32]
        wqT1 = qkv_w[32 * h1 : 32 * h1 + 32, :].T
        wq_rep = np.concatenate([wqT0, wqT0, wqT1, wqT1], axis=1).astype(BF16)
        wkT0 = qkv_w[128 + 32 * h0 : 128 + 32 * h0 + 32, :].T
        wkT1 = qkv_w[128 + 32 * h1 : 128 + 32 * h1 + 32, :].T
        wk_duo = np.concatenate([wkT0, wkT1], axis=1).astype(BF16)
        wvT0 = qkv_w[256 + 32 * h0 : 256 + 32 * h0 + 32, :].T
        wvT1 = qkv_w[256 + 32 * h1 : 256 + 32 * h1 + 32, :].T
        wv_duo = np.concatenate([wvT0, wvT1], axis=1).astype(BF16)
        wp_t = np.ascontiguousarray(proj_w[:, 64 * p : 64 * p + 64].T).astype(BF16)
        qb_rep = np.concatenate([
            qkv_b[32 * h0 : 32 * h0 + 32], qkv_b[32 * h0 : 32 * h0 + 32],
            qkv_b[32 * h1 : 32 * h1 + 32], qkv_b[32 * h1 : 32 * h1 + 32],
        ]).reshape(128, 1).astype(np.float32)
        kb_duo = np.concatenate([
            qkv_b[128 + 32 * h0 : 128 + 32 * h0 + 32],
            qkv_b[128 + 32 * h0 : 128 + 32 * h0 + 32],
            qkv_b[128 + 32 * h1 : 128 + 32 * h1 + 32],
            qkv_b[128 + 32 * h1 : 128 + 32 * h1 + 32],
        ]).reshape(128, 1).astype(np.float32)
        vb_duo = np.concatenate([
            qkv_b[256 + 32 * h0 : 256 + 32 * h0 + 32],
            qkv_b[256 + 32 * h1 : 256 + 32 * h1 + 32],
        ]).reshape(64, 1).astype(np.float32)
        oh_np = np.zeros((128, 8), dtype=np.float32)
        ohT_np = np.zeros((8, 128), dtype=np.float32)
        for g in range(8):
            oh_np[16 * g : 16 * g + 16, g] = 1.0 / 16.0
            ohT_np[g, 16 * g : 16 * g + 16] = 1.0
        in_maps.append({
            "x": xf[bi],
            "oh": oh_np, "ohT": ohT_np,
            "gnw": norm_w.reshape(128, 1).astype(np.float32),
            "gnb": norm_b.reshape(128, 1).astype(np.float32),
            "wq_rep": wq_rep, "wk_duo": wk_duo, "wv_duo": wv_duo, "wp_t": wp_t,
            "qb_rep": qb_rep, "kb_duo": kb_duo, "vb_duo": vb_duo,
        })
    return in_maps


def kernel(x, norm_w, norm_b, qkv_w, qkv_b, proj_w, proj_b):
    from concourse.bass_utils import run_bass_kernel_spmd

    _install_neff_cache()
    x = np.asarray(x, dtype=np.float32)
    norm_w = np.asarray(norm_w, dtype=np.float32)
    norm_b = np.asarray(norm_b, dtype=np.float32)
    qkv_w = np.asarray(qkv_w, dtype=np.float32)
    qkv_b = np.asarray(qkv_b, dtype=np.float32)
    proj_w = np.asarray(proj_w, dtype=np.float32)
    proj_b = np.asarray(proj_b, dtype=np.float32)

    b, c, hh, ww = x.shape
    T = hh * ww
    with_vbias = bool(np.any(qkv_b[256:]))
    nc = _get_nc(T, with_vbias)
    in_maps = _make_in_maps(x, norm_w, norm_b, qkv_w, qkv_b, proj_w)
    res = run_bass_kernel_spmd(nc, in_maps, core_ids=list(range(8)))
    parts = [res.results[core]["part"] for core in range(8)]
    out = x.reshape(b, c, T).copy()
    for bi in range(b):
        out[bi] += parts[2 * bi] + parts[2 * bi + 1]
    out += proj_b.reshape(1, c, 1)
    return out.reshape(b, c, hh, ww).astype(np.float32)


# revision 24
# speedup vs baseline: 92.4890x; 92.4890x over previous
"""Trainium2 Bass kernel for nn_AttentionBlock (GroupNorm + 4-head attention + proj + residual).

Sharding: 8 cores; core handles batch b = core//2 and head pair p = core%2
(global heads 2p, 2p+1). Each core computes GroupNorm(x_b), its heads' q/k/v,
full 4096x4096 attention for its 2 heads, and a partial projection output.
Host sums the two partial projections per batch element and adds the residual.

Layout strategy (channels-on-partitions throughout):
  xn   [c=128, T]      groupnormed input (bf16)
  Qrep [128, T]        Q^T replicated 2x per head: rows 32q hold head q//2 (bf16)
  Kq   [128, T/2]      K^T quadrant-packed: row 64*hl+32*(jc%2), col (jc//2)*128 (bf16)
  V'   [128, 65*JC]    per j-chunk [V_h0(32) | ones(1) | V_h1(32)] (bf16)
  S^T  psum [128,1024] 2-row-packed QK^T matmuls (K=32 contraction via tile_position)
  A^T  [128, 1024]     exp(scale*S^T) (ACT, bf16)
  O'   psum [33, 512]  (A@V)^T accumulated over j; extra ones-row = softmax denom r
  h    [64, T]         O'/r (bf16), both heads stacked
  part [128, T]        proj_w[:, 64p:64p+64] @ h  (fp32, DMA'd out)
"""

import numpy as np
import ml_dtypes

BF16 = ml_dtypes.bfloat16

_NC_CACHE = {}
_RUNNER_CACHE = {}


def _install_neff_cache():
    """Disk-cache walrus-compiled NEFFs keyed by BIR hash (speeds re-runs)."""
    import concourse.bass_utils as bu
    import concourse.bass2jax as b2j
    import hashlib, os, shutil

    if getattr(bu, "_neff_cache_installed", False):
        return
    orig = bu.compile_bir_kernel

    def cached(bir_json, tmpdir, neff_name="file.neff", **kw):
        data = bir_json if isinstance(bir_json, bytes) else bir_json.encode()
        h = hashlib.sha256(data).hexdigest()[:24]
        cdir = os.environ.get("BASS_NEFF_CACHE_DIR", "/tmp/bass_neff_cache")
        try:
            os.makedirs(cdir, exist_ok=True)
        except OSError:
            return orig(bir_json, tmpdir, neff_name=neff_name, **kw)
        cpath = os.path.join(cdir, h + ".neff")
        if os.path.exists(cpath):
            return cpath
        p = orig(bir_json, tmpdir, neff_name=neff_name, **kw)
        try:
            shutil.copy(p, cpath)
        except OSError:
            pass
        return p

    bu.compile_bir_kernel = cached
    b2j.compile_bir_kernel = cached
    bu._neff_cache_installed = True


def _build(T, with_vbias, repeats=1, loop=0, ablate=None, wide=False):
    import concourse.bacc as bacc
    import concourse.tile as tile
    from concourse import mybir

    f32 = mybir.dt.float32
    bf16 = mybir.dt.bfloat16
    IC = T // 512          # i chunks (512 wide)
    JC = T // 128          # j chunks (128 wide)
    JP = JC // 2           # packed j pairs
    KQCOL = (JC // 2) * 128  # = T/2
    NB = T // 512          # bn_stats chunks
    SCALE = 1.0 / np.sqrt(32.0)

    nc = bacc.Bacc("TRN2", target_bir_lowering=False, debug=False)

    x_d = nc.dram_tensor("x", [128, T], f32, kind="ExternalInput").ap()
    gnw_d = nc.dram_tensor("gnw", [128, 1], f32, kind="ExternalInput").ap()
    gnb_d = nc.dram_tensor("gnb", [128, 1], f32, kind="ExternalInput").ap()
    wq_d = nc.dram_tensor("wq_rep", [128, 128], bf16, kind="ExternalInput").ap()
    wk_d = nc.dram_tensor("wk_duo", [128, 64], bf16, kind="ExternalInput").ap()
    wv_d = nc.dram_tensor("wv_duo", [128, 64], bf16, kind="ExternalInput").ap()
    wp_d = nc.dram_tensor("wp_t", [64, 128], bf16, kind="ExternalInput").ap()
    qb_d = nc.dram_tensor("qb_rep", [128, 1], f32, kind="ExternalInput").ap()
    kb_d = nc.dram_tensor("kb_duo", [128, 1], f32, kind="ExternalInput").ap()
    vb_d = nc.dram_tensor("vb_duo", [64, 1], f32, kind="ExternalInput").ap()
    oh_d = nc.dram_tensor("oh", [128, 8], f32, kind="ExternalInput").ap()
    ohT_d = nc.dram_tensor("ohT", [8, 128], f32, kind="ExternalInput").ap()
    part_d = nc.dram_tensor("part", [128, T], f32, kind="ExternalOutput").ap()

    import contextlib

    with tile.TileContext(nc) as tc:
      with (tc.For_i(0, loop, 1) if loop else contextlib.nullcontext()):
       for _rep in range(repeats):
        # timing variant (repeats>1): chain rep N's input to rep N-1's output
        if _rep > 0:
            x_d = part_d.bitcast(f32)
        with (
            tc.tile_pool(name=f"consts{_rep}", bufs=1) as consts,
            tc.tile_pool(name=f"big{_rep}", bufs=1) as big,
            tc.tile_pool(name=f"at{_rep}", bufs=4) as atp,
            tc.tile_pool(name=f"tmp{_rep}", bufs=4) as tmp,
            tc.tile_pool(name=f"rb{_rep}", bufs=2) as rbp,
            tc.tile_pool(name=f"ps_c{_rep}", bufs=2, space="PSUM") as ps_c,
            tc.tile_pool(name=f"ps_st{_rep}", bufs=3, space="PSUM") as ps_st,
        ):
            # ---- constants / weights in ----
            gnw = consts.tile([128, 1], f32, tag="gnw")
            gnb = consts.tile([128, 1], f32, tag="gnb")
            wq = consts.tile([128, 128], bf16, tag="wq")
            wk = consts.tile([128, 64], bf16, tag="wk")
            wv = consts.tile([128, 64], bf16, tag="wv")
            wp = consts.tile([64, 128], bf16, tag="wp")
            qb = consts.tile([128, 1], f32, tag="qb")
            kb = consts.tile([128, 1], f32, tag="kb")
            vb = consts.tile([64, 1], f32, tag="vb")
            nc.sync.dma_start(out=gnw, in_=gnw_d)
            nc.sync.dma_start(out=gnb, in_=gnb_d)
            nc.sync.dma_start(out=wq, in_=wq_d)
            nc.sync.dma_start(out=wk, in_=wk_d)
            nc.sync.dma_start(out=wv, in_=wv_d)
            nc.sync.dma_start(out=wp, in_=wp_d)
            nc.sync.dma_start(out=qb, in_=qb_d)
            nc.sync.dma_start(out=kb, in_=kb_d)
            nc.sync.dma_start(out=vb, in_=vb_d)

            eps_t = consts.tile([128, 1], f32, tag="eps")
            nc.vector.memset(eps_t, 1e-5)
            ones32 = consts.tile([1, 32], bf16, tag="ones32")
            nc.vector.memset(ones32, 1.0)
            # group one-hot (value 1/16 folds in the group-average) and its transpose
            oh = consts.tile([128, 8], f32, tag="oh")
            nc.sync.dma_start(out=oh, in_=oh_d)
            ohT = consts.tile([8, 128], f32, tag="ohT")
            nc.sync.dma_start(out=ohT, in_=ohT_d)

            # ---- x in ----
            x_sb = big.tile([128, T], f32, tag="x")
            nc.sync.dma_start(out=x_sb, in_=x_d)

            # ---- GroupNorm stats ----
            stats = tmp.tile([128, NB, 6], f32, tag="bnst")
            for i in range(NB):
                nc.vector.bn_stats(out=stats[:, i, :], in_=x_sb[:, 512 * i : 512 * i + 512])
            mv = tmp.tile([128, 2], f32, tag="mv")
            nc.vector.bn_aggr(out=mv, in_=stats)
            # per-channel (mean, E[x^2])
            cs = tmp.tile([128, 2], f32, tag="cs")
            nc.vector.tensor_copy(cs[:, 0:1], mv[:, 0:1])
            msq = tmp.tile([128, 1], f32, tag="msq")
            nc.vector.tensor_mul(msq, mv[:, 0:1], mv[:, 0:1])
            nc.vector.tensor_add(cs[:, 1:2], mv[:, 1:2], msq)
            # group-average via one-hot matmul (fp32), then broadcast back
            gs_ps = ps_c.tile([8, 2], f32, tag="c")
            nc.tensor.matmul(gs_ps, oh, cs)
            gs_sb = tmp.tile([8, 2], f32, tag="gs")
            nc.vector.tensor_copy(gs_sb, gs_ps)
            cb_ps = ps_c.tile([128, 2], f32, tag="c")
            nc.tensor.matmul(cb_ps, ohT, gs_sb)
            cb = tmp.tile([128, 2], f32, tag="cb")
            nc.vector.tensor_copy(cb, cb_ps)
            # rstd = 1/sqrt(E[x^2] - mean^2 + eps)
            gmsq = tmp.tile([128, 1], f32, tag="gmsq")
            nc.vector.tensor_mul(gmsq, cb[:, 0:1], cb[:, 0:1])
            gvar = tmp.tile([128, 1], f32, tag="gvar")
            nc.vector.tensor_sub(gvar, cb[:, 1:2], gmsq)
            nc.scalar.activation(
                out=gvar, in_=gvar, func=mybir.ActivationFunctionType.Sqrt,
                bias=eps_t, scale=1.0,
            )
            rstd = tmp.tile([128, 1], f32, tag="rstd")
            nc.vector.reciprocal(out=rstd, in_=gvar)
            # affine: xn = x * (norm_w * rstd) + (norm_b - mean * norm_w * rstd)
            s_ap = tmp.tile([128, 1], f32, tag="s_ap")
            nc.vector.tensor_mul(s_ap, gnw, rstd)
            mb = tmp.tile([128, 1], f32, tag="mb")
            nc.vector.tensor_mul(mb, cb[:, 0:1], s_ap)
            b_ap = tmp.tile([128, 1], f32, tag="b_ap")
            nc.vector.tensor_sub(b_ap, gnb, mb)
            xn = big.tile([128, T], bf16, tag="xn")
            nc.vector.tensor_scalar(
                out=xn, in0=x_sb, scalar1=s_ap, scalar2=b_ap,
                op0=mybir.AluOpType.mult, op1=mybir.AluOpType.add,
            )

            # ---- QKV ----
            q_rep = big.tile([128, T], bf16, tag="q_rep")
            for i in range(IC):
                qp = ps_c.tile([128, 512], f32, tag="c")
                nc.tensor.matmul(qp, wq, xn[:, 512 * i : 512 * i + 512])
                nc.vector.tensor_scalar(
                    out=q_rep[:, 512 * i : 512 * i + 512], in0=qp,
                    scalar1=qb, scalar2=None, op0=mybir.AluOpType.add,
                )

            k_q = big.tile([128, KQCOL], bf16, tag="k_q")
            RW = min(512, KQCOL)
            for t in range((KQCOL + 511) // 512):
                kp = ps_c.tile([128, RW], f32, tag="c")
                for hl in range(2):
                    for jj in range(min(8, JC - 8 * t)):
                        jc = 8 * t + jj
                        nc.tensor.matmul(
                            kp[64 * hl + 32 * (jc % 2) : 64 * hl + 32 * (jc % 2) + 32,
                               ((jc // 2) % 4) * 128 : ((jc // 2) % 4) * 128 + 128],
                            wk[:, 32 * hl : 32 * hl + 32],
                            xn[:, 128 * jc : 128 * jc + 128],
                            tile_position=(0, 64 * hl + 32 * (jc % 2)),
                            skip_group_check=True,
                        )
                nc.vector.tensor_scalar(
                    out=k_q[:, 512 * t : 512 * t + RW], in0=kp,
                    scalar1=kb, scalar2=None, op0=mybir.AluOpType.add,
                )

            # per j-chunk block: [V_h0(0:32) | ones(32) | V_h1(33:65) | ones(65)]
            v_sb = big.tile([128, 66 * JC], bf16, tag="v_sb")
            nc.vector.memset(v_sb[:, 32 : 32 + 66 * (JC - 1) + 1 : 66], 1.0)
            nc.vector.memset(v_sb[:, 65 : 65 + 66 * (JC - 1) + 1 : 66], 1.0)
            for jc in range(JC):
                vp = ps_c.tile([128, 64], f32, tag="c")
                nc.tensor.matmul(vp, xn[:, 128 * jc : 128 * jc + 128], wv)
                nc.vector.tensor_copy(v_sb[:, 66 * jc : 66 * jc + 32], vp[:, 0:32])
                nc.vector.tensor_copy(v_sb[:, 66 * jc + 33 : 66 * jc + 65], vp[:, 32:64])

            # ---- attention ----
            # Round (ic, jp): 4 QK matmuls 4-way row-packed (head hl, j parity
            # g at PE row group 32*(2hl+g)), 2 exp activations, 4 AV matmuls
            # col-packed (h0 -> O' partitions 0:33 col group 0, h1 -> 64:97
            # col group 64; one PSUM bank for both heads). AV lags one round
            # so the in-order PE stream never stalls on this round's exp.
            h_sb = big.tile([64, T], bf16, tag="h_sb")
            for i in range(IC):
                op = ps_c.tile([97, 512], f32, tag="c")
                # h1's first AV matmul uses start=False (h0's start already
                # cleared the bank's has_written bits) - zero its region so
                # sim and hw agree on the overwrite-vs-accumulate base.
                nc.vector.memset(op[64:97, :], 0.0)

                def _emit_av(at_, jp_):
                    for g in range(2):
                        jc = 2 * jp_ + g
                        for hl in range(2):
                            nc.tensor.matmul(
                                op[64 * hl : 64 * hl + 33, :],
                                v_sb[:, 66 * jc + 33 * hl : 66 * jc + 33 * hl + 33],
                                at_[hl][:, 512 * g : 512 * g + 512],
                                start=(jp_ == 0 and g == 0 and hl == 0),
                                stop=(jp_ == JP - 1 and g == 1),
                                tile_position=(0, 64 * hl),
                                skip_group_check=True,
                            )

                prev_at, prev_jp = None, None
                for jp in range(JP):
                    st = {}
                    for hl in range(2):
                        st_t = ps_st.tile([128, 1024], f32, tag="st")
                        st[hl] = st_t
                        for g in range(2):
                            jc = 2 * jp + g
                            q = 2 * hl + g
                            nc.tensor.matmul(
                                st[hl][:, 512 * g : 512 * g + 512],
                                k_q[32 * q : 32 * q + 32,
                                    ((jc // 2) % 4) * 128 + 512 * (jc // 8) : ((jc // 2) % 4) * 128 + 512 * (jc // 8) + 128],
                                q_rep[32 * q : 32 * q + 32, 512 * i : 512 * i + 512],
                                tile_position=(32 * q, 0),
                                skip_group_check=True,
                            )
                    at = {}
                    for hl in range(2):
                        at_t = atp.tile([128, 1024], bf16, tag="at")
                        at[hl] = at_t
                        nc.scalar.activation(
                            out=at[hl], in_=st[hl],
                            func=mybir.ActivationFunctionType.Exp, scale=SCALE,
                        )
                    if prev_at is not None:
                        _emit_av(prev_at, prev_jp)
                    prev_at, prev_jp = at, jp
                _emit_av(prev_at, prev_jp)
                # normalize: h = O'[d] / r  (r is each head's ones-column row)
                rb_ps = ps_c.tile([97, 512], f32, tag="c")
                for hl in range(2):
                    rinv = tmp.tile([1, 512], bf16, tag="rinv")
                    with nc.allow_low_precision(reason="softmax denom fits bf16"):
                        nc.vector.reciprocal(out=rinv, in_=op[64 * hl + 32 : 64 * hl + 33, :])
                    nc.tensor.matmul(
                        rb_ps[64 * hl : 64 * hl + 32, :], ones32, rinv,
                        tile_position=(0, 64 * hl), skip_group_check=True,
                    )
                for hl in range(2):
                    rb_sb = rbp.tile([32, 512], bf16, tag="rb")
                    nc.vector.tensor_copy(rb_sb, rb_ps[64 * hl : 64 * hl + 32, :])
                    h_slice = h_sb[32 * hl : 32 * hl + 32, 512 * i : 512 * i + 512]
                    nc.vector.tensor_mul(h_slice, op[64 * hl : 64 * hl + 32, :], rb_sb)
                    if with_vbias:
                        nc.vector.tensor_scalar(
                            out=h_slice, in0=h_slice,
                            scalar1=vb[32 * hl : 32 * hl + 32, :], scalar2=None,
                            op0=mybir.AluOpType.add,
                        )

            # ---- proj ----
            out_sb = big.tile([128, T], f32, tag="out_sb")
            for i in range(IC):
                pp = ps_c.tile([128, 512], f32, tag="c")
                nc.tensor.matmul(pp, wp, h_sb[:, 512 * i : 512 * i + 512])
                nc.vector.tensor_copy(out_sb[:, 512 * i : 512 * i + 512], pp)
            nc.sync.dma_start(out=part_d, in_=out_sb)

    nc.compile()
    return nc


def _get_nc(T, with_vbias):
    key = (T, with_vbias)
    if key not in _NC_CACHE:
        _NC_CACHE[key] = _build(T, with_vbias)
    return _NC_CACHE[key]


def _make_in_maps(x, norm_w, norm_b, qkv_w, qkv_b, proj_w):
    b, c, hh, ww = x.shape
    T = hh * ww
    xf = np.ascontiguousarray(x.reshape(b, c, T), dtype=np.float32)
    in_maps = []
    for core in range(8):
        p = core % 2
        bi = core // 2
        h0, h1 = 2 * p, 2 * p + 1
        wqT0 = qkv_w[32 * h0 : 32 * h0 + 32, :].T  # [128, 32]
        wqT1 = qkv_w[32 * h1 : 32 * h1 + 32, :].T
        wq_rep = np.concatenate([wqT0, wqT0, wqT1, wqT1], axis=1).astype(BF16)
        wkT0 = qkv_w[128 + 32 * h0 : 128 + 32 * h0 + 32, :].T
        wkT1 = qkv_w[128 + 32 * h1 : 128 + 32 * h1 + 32, :].T
        wk_duo = np.concatenate([wkT0, wkT1], axis=1).astype(BF16)
        wvT0 = qkv_w[256 + 32 * h0 : 256 + 32 * h0 + 32, :].T
        wvT1 = qkv_w[256 + 32 * h1 : 256 + 32 * h1 + 32, :].T
        wv_duo = np.concatenate([wvT0, wvT1], axis=1).astype(BF16)
        wp_t = np.ascontiguousarray(proj_w[:, 64 * p : 64 * p + 64].T).astype(BF16)
        qb_rep = np.concatenate([
            qkv_b[32 * h0 : 32 * h0 + 32], qkv_b[32 * h0 : 32 * h0 + 32],
            qkv_b[32 * h1 : 32 * h1 + 32], qkv_b[32 * h1 : 32 * h1 + 32],
        ]).reshape(128, 1).astype(np.float32)
        kb_duo = np.concatenate([
            qkv_b[128 + 32 * h0 : 128 + 32 * h0 + 32],
            qkv_b[128 + 32 * h0 : 128 + 32 * h0 + 32],
            qkv_b[128 + 32 * h1 : 128 + 32 * h1 + 32],
            qkv_b[128 + 32 * h1 : 128 + 32 * h1 + 32],
        ]).reshape(128, 1).astype(np.float32)
        vb_duo = np.concatenate([
            qkv_b[256 + 32 * h0 : 256 + 32 * h0 + 32],
            qkv_b[256 + 32 * h1 : 256 + 32 * h1 + 32],
        ]).reshape(64, 1).astype(np.float32)
        oh_np = np.zeros((128, 8), dtype=np.float32)
        ohT_np = np.zeros((8, 128), dtype=np.float32)
        for g in range(8):
            oh_np[16 * g : 16 * g + 16, g] = 1.0 / 16.0
            ohT_np[g, 16 * g : 16 * g + 16] = 1.0
        in_maps.append({
            "x": xf[bi],
            "oh": oh_np, "ohT": ohT_np,
            "gnw": norm_w.reshape(128, 1).astype(np.float32),
            "gnb": norm_b.reshape(128, 1).astype(np.float32),
            "wq_rep": wq_rep, "wk_duo": wk_duo, "wv_duo": wv_duo, "wp_t": wp_t,
            "qb_rep": qb_rep, "kb_duo": kb_duo, "vb_duo": vb_duo,
        })
    return in_maps


def kernel(x, norm_w, norm_b, qkv_w, qkv_b, proj_w, proj_b):
    from concourse.bass_utils import run_bass_kernel_spmd

    _install_neff_cache()
    x = np.asarray(x, dtype=np.float32)
    norm_w = np.asarray(norm_w, dtype=np.float32)
    norm_b = np.asarray(norm_b, dtype=np.float32)
    qkv_w = np.asarray(qkv_w, dtype=np.float32)
    qkv_b = np.asarray(qkv_b, dtype=np.float32)
    proj_w = np.asarray(proj_w, dtype=np.float32)
    proj_b = np.asarray(proj_b, dtype=np.float32)

    b, c, hh, ww = x.shape
    T = hh * ww
    with_vbias = bool(np.any(qkv_b[256:]))
    nc = _get_nc(T, with_vbias)
    in_maps = _make_in_maps(x, norm_w, norm_b, qkv_w, qkv_b, proj_w)
    res = run_bass_kernel_spmd(nc, in_maps, core_ids=list(range(8)))
    parts = [res.results[core]["part"] for core in range(8)]
    out = x.reshape(b, c, T).copy()
    for bi in range(b):
        out[bi] += parts[2 * bi] + parts[2 * bi + 1]
    out += proj_b.reshape(1, c, 1)
    return out.reshape(b, c, hh, ww).astype(np.float32)


# revision 26
# speedup vs baseline: 93.7342x; 1.0135x over previous
"""Trainium2 Bass kernel for nn_AttentionBlock (GroupNorm + 4-head attention + proj + residual).

Sharding: 8 cores; core handles batch b = core//2 and head pair p = core%2
(global heads 2p, 2p+1). Each core computes GroupNorm(x_b), its heads' q/k/v,
full 4096x4096 attention for its 2 heads, and a partial projection output.
Host sums the two partial projections per batch element and adds the residual.

Layout strategy (channels-on-partitions throughout):
  xn   [c=128, T]      groupnormed input (bf16)
  Qrep [128, T]        Q^T replicated 2x per head: rows 32q hold head q//2 (bf16)
  Kq   [128, T/2]      K^T quadrant-packed: row 64*hl+32*(jc%2), col (jc//2)*128 (bf16)
  V'   [128, 65*JC]    per j-chunk [V_h0(32) | ones(1) | V_h1(32)] (bf16)
  S^T  psum [128,1024] 2-row-packed QK^T matmuls (K=32 contraction via tile_position)
  A^T  [128, 1024]     exp(scale*S^T) (ACT, bf16)
  O'   psum [33, 512]  (A@V)^T accumulated over j; extra ones-row = softmax denom r
  h    [64, T]         O'/r (bf16), both heads stacked
  part [128, T]        proj_w[:, 64p:64p+64] @ h  (fp32, DMA'd out)
"""

import numpy as np
import ml_dtypes

BF16 = ml_dtypes.bfloat16

_NC_CACHE = {}
_RUNNER_CACHE = {}


def _install_neff_cache():
    """Disk-cache walrus-compiled NEFFs keyed by BIR hash (speeds re-runs)."""
    import concourse.bass_utils as bu
    import concourse.bass2jax as b2j
    import hashlib, os, shutil

    if getattr(bu, "_neff_cache_installed", False):
        return
    orig = bu.compile_bir_kernel

    def cached(bir_json, tmpdir, neff_name="file.neff", **kw):
        data = bir_json if isinstance(bir_json, bytes) else bir_json.encode()
        h = hashlib.sha256(data).hexdigest()[:24]
        cdir = os.environ.get("BASS_NEFF_CACHE_DIR", "/tmp/bass_neff_cache")
        try:
            os.makedirs(cdir, exist_ok=True)
        except OSError:
            return orig(bir_json, tmpdir, neff_name=neff_name, **kw)
        cpath = os.path.join(cdir, h + ".neff")
        if os.path.exists(cpath):
            return cpath
        p = orig(bir_json, tmpdir, neff_name=neff_name, **kw)
        try:
            shutil.copy(p, cpath)
        except OSError:
            pass
        return p

    bu.compile_bir_kernel = cached
    b2j.compile_bir_kernel = cached
    bu._neff_cache_installed = True


def _build(T, with_vbias, repeats=1, loop=0, ablate=None, wide=False):
    import concourse.bacc as bacc
    import concourse.tile as tile
    from concourse import mybir

    f32 = mybir.dt.float32
    bf16 = mybir.dt.bfloat16
    IC = T // 512          # i chunks (512 wide)
    JC = T // 128          # j chunks (128 wide)
    JP = JC // 2           # packed j pairs
    KQCOL = (JC // 2) * 128  # = T/2
    NB = T // 512          # bn_stats chunks
    SCALE = 1.0 / np.sqrt(32.0)

    nc = bacc.Bacc("TRN2", target_bir_lowering=False, debug=False)

    x_d = nc.dram_tensor("x", [128, T], f32, kind="ExternalInput").ap()
    gnw_d = nc.dram_tensor("gnw", [128, 1], f32, kind="ExternalInput").ap()
    gnb_d = nc.dram_tensor("gnb", [128, 1], f32, kind="ExternalInput").ap()
    wq_d = nc.dram_tensor("wq_rep", [128, 128], bf16, kind="ExternalInput").ap()
    wk_d = nc.dram_tensor("wk_duo", [128, 64], bf16, kind="ExternalInput").ap()
    wv_d = nc.dram_tensor("wv_duo", [128, 64], bf16, kind="ExternalInput").ap()
    wp_d = nc.dram_tensor("wp_t", [64, 128], bf16, kind="ExternalInput").ap()
    qb_d = nc.dram_tensor("qb_rep", [128, 1], f32, kind="ExternalInput").ap()
    kb_d = nc.dram_tensor("kb_duo", [128, 1], f32, kind="ExternalInput").ap()
    vb_d = nc.dram_tensor("vb_duo", [64, 1], f32, kind="ExternalInput").ap()
    oh_d = nc.dram_tensor("oh", [128, 8], f32, kind="ExternalInput").ap()
    ohT_d = nc.dram_tensor("ohT", [8, 128], f32, kind="ExternalInput").ap()
    part_d = nc.dram_tensor("part", [128, T], f32, kind="ExternalOutput").ap()

    import contextlib

    with tile.TileContext(nc) as tc:
      with (tc.For_i(0, loop, 1) if loop else contextlib.nullcontext()):
       for _rep in range(repeats):
        # timing variant (repeats>1): chain rep N's input to rep N-1's output
        if _rep > 0:
            x_d = part_d.bitcast(f32)
        with (
            tc.tile_pool(name=f"consts{_rep}", bufs=1) as consts,
            tc.tile_pool(name=f"big{_rep}", bufs=1) as big,
            tc.tile_pool(name=f"at{_rep}", bufs=4) as atp,
            tc.tile_pool(name=f"tmp{_rep}", bufs=4) as tmp,
            tc.tile_pool(name=f"rb{_rep}", bufs=2) as rbp,
            tc.tile_pool(name=f"ps_c{_rep}", bufs=2, space="PSUM") as ps_c,
            tc.tile_pool(name=f"ps_st{_rep}", bufs=3, space="PSUM") as ps_st,
        ):
            # ---- constants / weights in ----
            gnw = consts.tile([128, 1], f32, tag="gnw")
            gnb = consts.tile([128, 1], f32, tag="gnb")
            wq = consts.tile([128, 128], bf16, tag="wq")
            wk = consts.tile([128, 64], bf16, tag="wk")
            wv = consts.tile([128, 64], bf16, tag="wv")
            wp = consts.tile([64, 128], bf16, tag="wp")
            qb = consts.tile([128, 1], f32, tag="qb")
            kb = consts.tile([128, 1], f32, tag="kb")
            vb = consts.tile([64, 1], f32, tag="vb")
            nc.sync.dma_start(out=gnw, in_=gnw_d)
            nc.sync.dma_start(out=gnb, in_=gnb_d)
            nc.sync.dma_start(out=wq, in_=wq_d)
            nc.sync.dma_start(out=wk, in_=wk_d)
            nc.sync.dma_start(out=wv, in_=wv_d)
            nc.sync.dma_start(out=wp, in_=wp_d)
            nc.sync.dma_start(out=qb, in_=qb_d)
            nc.sync.dma_start(out=kb, in_=kb_d)
            nc.sync.dma_start(out=vb, in_=vb_d)

            eps_t = consts.tile([128, 1], f32, tag="eps")
            nc.vector.memset(eps_t, 1e-5)
            ones32 = consts.tile([1, 32], bf16, tag="ones32")
            nc.vector.memset(ones32, 1.0)
            # group one-hot (value 1/16 folds in the group-average) and its transpose
            oh = consts.tile([128, 8], f32, tag="oh")
            nc.sync.dma_start(out=oh, in_=oh_d)
            ohT = consts.tile([8, 128], f32, tag="ohT")
            nc.sync.dma_start(out=ohT, in_=ohT_d)

            # ---- x in ----
            x_sb = big.tile([128, T], f32, tag="x")
            nc.sync.dma_start(out=x_sb, in_=x_d)

            # ---- GroupNorm stats ----
            stats = tmp.tile([128, NB, 6], f32, tag="bnst")
            for i in range(NB):
                nc.vector.bn_stats(out=stats[:, i, :], in_=x_sb[:, 512 * i : 512 * i + 512])
            mv = tmp.tile([128, 2], f32, tag="mv")
            nc.vector.bn_aggr(out=mv, in_=stats)
            # per-channel (mean, E[x^2])
            cs = tmp.tile([128, 2], f32, tag="cs")
            nc.vector.tensor_copy(cs[:, 0:1], mv[:, 0:1])
            msq = tmp.tile([128, 1], f32, tag="msq")
            nc.vector.tensor_mul(msq, mv[:, 0:1], mv[:, 0:1])
            nc.vector.tensor_add(cs[:, 1:2], mv[:, 1:2], msq)
            # group-average via one-hot matmul (fp32), then broadcast back
            gs_ps = ps_c.tile([8, 2], f32, tag="c")
            nc.tensor.matmul(gs_ps, oh, cs)
            gs_sb = tmp.tile([8, 2], f32, tag="gs")
            nc.vector.tensor_copy(gs_sb, gs_ps)
            cb_ps = ps_c.tile([128, 2], f32, tag="c")
            nc.tensor.matmul(cb_ps, ohT, gs_sb)
            cb = tmp.tile([128, 2], f32, tag="cb")
            nc.vector.tensor_copy(cb, cb_ps)
            # rstd = 1/sqrt(E[x^2] - mean^2 + eps)
            gmsq = tmp.tile([128, 1], f32, tag="gmsq")
            nc.vector.tensor_mul(gmsq, cb[:, 0:1], cb[:, 0:1])
            gvar = tmp.tile([128, 1], f32, tag="gvar")
            nc.vector.tensor_sub(gvar, cb[:, 1:2], gmsq)
            nc.scalar.activation(
                out=gvar, in_=gvar, func=mybir.ActivationFunctionType.Sqrt,
                bias=eps_t, scale=1.0,
            )
            rstd = tmp.tile([128, 1], f32, tag="rstd")
            nc.vector.reciprocal(out=rstd, in_=gvar)
            # affine: xn = x * (norm_w * rstd) + (norm_b - mean * norm_w * rstd)
            s_ap = tmp.tile([128, 1], f32, tag="s_ap")
            nc.vector.tensor_mul(s_ap, gnw, rstd)
            mb = tmp.tile([128, 1], f32, tag="mb")
            nc.vector.tensor_mul(mb, cb[:, 0:1], s_ap)
            b_ap = tmp.tile([128, 1], f32, tag="b_ap")
            nc.vector.tensor_sub(b_ap, gnb, mb)
            xn = big.tile([128, T], bf16, tag="xn")
            nc.vector.tensor_scalar(
                out=xn, in0=x_sb, scalar1=s_ap, scalar2=b_ap,
                op0=mybir.AluOpType.mult, op1=mybir.AluOpType.add,
            )

            # ---- QKV ----
            q_rep = big.tile([128, T], bf16, tag="q_rep")
            for i in range(IC):
                qp = ps_c.tile([128, 512], f32, tag="c")
                nc.tensor.matmul(qp, wq, xn[:, 512 * i : 512 * i + 512])
                nc.vector.tensor_scalar(
                    out=q_rep[:, 512 * i : 512 * i + 512], in0=qp,
                    scalar1=qb, scalar2=None, op0=mybir.AluOpType.add,
                )

            k_q = big.tile([128, KQCOL], bf16, tag="k_q")
            RW = min(512, KQCOL)
            for t in range((KQCOL + 511) // 512):
                kp = ps_c.tile([128, RW], f32, tag="c")
                for hl in range(2):
                    for jj in range(min(8, JC - 8 * t)):
                        jc = 8 * t + jj
                        nc.tensor.matmul(
                            kp[64 * hl + 32 * (jc % 2) : 64 * hl + 32 * (jc % 2) + 32,
                               ((jc // 2) % 4) * 128 : ((jc // 2) % 4) * 128 + 128],
                            wk[:, 32 * hl : 32 * hl + 32],
                            xn[:, 128 * jc : 128 * jc + 128],
                            tile_position=(0, 64 * hl + 32 * (jc % 2)),
                            skip_group_check=True,
                        )
                nc.vector.tensor_scalar(
                    out=k_q[:, 512 * t : 512 * t + RW], in0=kp,
                    scalar1=kb, scalar2=None, op0=mybir.AluOpType.add,
                )

            # per j-chunk block: [V_h0(0:32) | ones(32) | V_h1(33:65) | ones(65)]
            v_sb = big.tile([128, 66 * JC], bf16, tag="v_sb")
            nc.vector.memset(v_sb[:, 32 : 32 + 66 * (JC - 1) + 1 : 66], 1.0)
            nc.vector.memset(v_sb[:, 65 : 65 + 66 * (JC - 1) + 1 : 66], 1.0)
            for jc in range(JC):
                vp = ps_c.tile([128, 64], f32, tag="c")
                nc.tensor.matmul(vp, xn[:, 128 * jc : 128 * jc + 128], wv)
                nc.vector.tensor_copy(v_sb[:, 66 * jc : 66 * jc + 32], vp[:, 0:32])
                nc.vector.tensor_copy(v_sb[:, 66 * jc + 33 : 66 * jc + 65], vp[:, 32:64])

            # ---- attention ----
            # Round (ic, jp): 4 QK matmuls 4-way row-packed (head hl, j parity
            # g at PE row group 32*(2hl+g)), 2 exp activations, 4 AV matmuls
            # col-packed (h0 -> O' partitions 0:33 col group 0, h1 -> 64:97
            # col group 64; one PSUM bank for both heads). AV lags one round
            # so the in-order PE stream never stalls on this round's exp.
            h_sb = big.tile([64, T], bf16, tag="h_sb")
            for i in range(IC):
                op = ps_c.tile([97, 512], f32, tag="c")
                # h1's first AV matmul uses start=False (h0's start already
                # cleared the bank's has_written bits) - zero its region so
                # sim and hw agree on the overwrite-vs-accumulate base.
                nc.vector.memset(op[64:97, :], 0.0)

                def _emit_av(at_, jp_):
                    for g in range(2):
                        jc = 2 * jp_ + g
                        for hl in range(2):
                            nc.tensor.matmul(
                                op[64 * hl : 64 * hl + 33, :],
                                v_sb[:, 66 * jc + 33 * hl : 66 * jc + 33 * hl + 33],
                                at_[hl][:, 512 * g : 512 * g + 512],
                                start=(jp_ == 0 and g == 0 and hl == 0),
                                stop=(jp_ == JP - 1 and g == 1),
                                tile_position=(0, 64 * hl),
                                skip_group_check=True,
                            )

                prev_at, prev_jp = None, None
                for jp in range(JP):
                    st = {}
                    for hl in range(2):
                        st_t = ps_st.tile([128, 1024], f32, tag="st")
                        st[hl] = st_t
                        for g in range(2):
                            jc = 2 * jp + g
                            q = 2 * hl + g
                            nc.tensor.matmul(
                                st[hl][:, 512 * g : 512 * g + 512],
                                k_q[32 * q : 32 * q + 32,
                                    ((jc // 2) % 4) * 128 + 512 * (jc // 8) : ((jc // 2) % 4) * 128 + 512 * (jc // 8) + 128],
                                q_rep[32 * q : 32 * q + 32, 512 * i : 512 * i + 512],
                                tile_position=(32 * q, 0),
                                skip_group_check=True,
                            )
                    at = {}
                    for hl in range(2):
                        at_t = atp.tile([128, 1024], bf16, tag="at")
                        at[hl] = at_t
                        nc.scalar.activation(
                            out=at[hl], in_=st[hl],
                            func=mybir.ActivationFunctionType.Exp, scale=SCALE,
                        )
                    if prev_at is not None:
                        _emit_av(prev_at, prev_jp)
                    prev_at, prev_jp = at, jp
                _emit_av(prev_at, prev_jp)
                # normalize: h = O'[d] / r  (r is each head's ones-column row)
                rb_ps = ps_c.tile([97, 512], f32, tag="c")
                for hl in range(2):
                    rinv = tmp.tile([1, 512], bf16, tag="rinv")
                    with nc.allow_low_precision(reason="softmax denom fits bf16"):
                        nc.vector.reciprocal(out=rinv, in_=op[64 * hl + 32 : 64 * hl + 33, :])
                    nc.tensor.matmul(
                        rb_ps[64 * hl : 64 * hl + 32, :], ones32, rinv,
                        tile_position=(0, 64 * hl), skip_group_check=True,
                    )
                for hl in range(2):
                    rb_sb = rbp.tile([32, 512], bf16, tag="rb")
                    nc.vector.tensor_copy(rb_sb, rb_ps[64 * hl : 64 * hl + 32, :])
                    h_slice = h_sb[32 * hl : 32 * hl + 32, 512 * i : 512 * i + 512]
                    nc.vector.tensor_mul(h_slice, op[64 * hl : 64 * hl + 32, :], rb_sb)
                    if with_vbias:
                        nc.vector.tensor_scalar(
                            out=h_slice, in0=h_slice,
                            scalar1=vb[32 * hl : 32 * hl + 32, :], scalar2=None,
                            op0=mybir.AluOpType.add,
                        )

            # ---- proj ----
            out_sb = big.tile([128, T], f32, tag="out_sb")
            for i in range(IC):
                pp = ps_c.tile([128, 512], f32, tag="c")
                nc.tensor.matmul(pp, wp, h_sb[:, 512 * i : 512 * i + 512])
                nc.vector.tensor_copy(out_sb[:, 512 * i : 512 * i + 512], pp)
            nc.sync.dma_start(out=part_d, in_=out_sb)

    nc.compile()
    return nc


def _get_nc(T, with_vbias):
    key = (T, with_vbias)
    if key not in _NC_CACHE:
        _NC_CACHE[key] = _build(T, with_vbias)
    return _NC_CACHE[key]


def _make_in_maps(x, norm_w, norm_b, qkv_w, qkv_b, proj_w):
    b, c, hh, ww = x.shape
    T = hh * ww
    xf = np.ascontiguousarray(x.reshape(b, c, T), dtype=np.float32)
    in_maps = []
    for core in range(8):
        p = core % 2
        bi = core // 2
        h0, h1 = 2 * p, 2 * p + 1
        wqT0 = qkv_w[32 * h0 : 32 * h0 + 32, :].T  # [128, 32]
        wqT1 = qkv_w[32 * h1 : 32 * h1 + 32, :].T
        wq_rep = np.concatenate([wqT0, wqT0, wqT1, wqT1], axis=1).astype(BF16)
        wkT0 = qkv_w[128 + 32 * h0 : 128 + 32 * h0 + 32, :].T
        wkT1 = qkv_w[128 + 32 * h1 : 128 + 32 * h1 + 32, :].T
        wk_duo = np.concatenate([wkT0, wkT1], axis=1).astype(BF16)
        wvT0 = qkv_w[256 + 32 * h0 : 256 + 32 * h0 + 32, :].T
        wvT1 = qkv_w[256 + 32 * h1 : 256 + 32 * h1 + 32, :].T
        wv_duo = np.concatenate([wvT0, wvT1], axis=1).astype(BF16)
        wp_t = np.ascontiguousarray(proj_w[:, 64 * p : 64 * p + 64].T).astype(BF16)
        qb_rep = np.concatenate([
            qkv_b[32 * h0 : 32 * h0 + 32], qkv_b[32 * h0 : 32 * h0 + 32],
            qkv_b[32 * h1 : 32 * h1 + 32], qkv_b[32 * h1 : 32 * h1 + 32],
        ]).reshape(128, 1).astype(np.float32)
        kb_duo = np.concatenate([
            qkv_b[128 + 32 * h0 : 128 + 32 * h0 + 32],
            qkv_b[128 + 32 * h0 : 128 + 32 * h0 + 32],
            qkv_b[128 + 32 * h1 : 128 + 32 * h1 + 32],
            qkv_b[128 + 32 * h1 : 128 + 32 * h1 + 32],
        ]).reshape(128, 1).astype(np.float32)
        vb_duo = np.concatenate([
            qkv_b[256 + 32 * h0 : 256 + 32 * h0 + 32],
            qkv_b[256 + 32 * h1 : 256 + 32 * h1 + 32],
        ]).reshape(64, 1).astype(np.float32)
        oh_np = np.zeros((128, 8), dtype=np.float32)
        ohT_np = np.zeros((8, 128), dtype=np.float32)
        for g in range(8):
            oh_np[16 * g : 16 * g + 16, g] = 1.0 / 16.0
            ohT_np[g, 16 * g : 16 * g + 16] = 1.0
        in_maps.append({
            "x": xf[bi],
            "oh": oh_np, "ohT": ohT_np,
            "gnw": norm_w.reshape(128, 1).astype(np.float32),
            "gnb": norm_b.reshape(128, 1).astype(np.float32),
            "wq_rep": wq_rep, "wk_duo": wk_duo, "wv_duo": wv_duo, "wp_t": wp_t,
            "qb_rep": qb_rep, "kb_duo": kb_duo, "vb_duo": vb_duo,
        })
    return in_maps


def kernel(x, norm_w, norm_b, qkv_w, qkv_b, proj_w, proj_b):
    from concourse.bass_utils import run_bass_kernel_spmd

    _install_neff_cache()
    x = np.asarray(x, dtype=np.float32)
    norm_w = np.asarray(norm_w, dtype=np.float32)
    norm_b = np.asarray(norm_b, dtype=np.float32)
    qkv_w = np.asarray(qkv_w, dtype=np.float32)
    qkv_b = np.asarray(qkv_b, dtype=np.float32)
    proj_w = np.asarray(proj_w, dtype=np.float32)
    proj_b = np.asarray(proj_b, dtype=np.float32)

    b, c, hh, ww = x.shape
    T = hh * ww
    with_vbias = bool(np.any(qkv_b[256:]))
    nc = _get_nc(T, with_vbias)
    in_maps = _make_in_maps(x, norm_w, norm_b, qkv_w, qkv_b, proj_w)
    res = run_bass_kernel_spmd(nc, in_maps, core_ids=list(range(8)))
    parts = [res.results[core]["part"] for core in range(8)]
    out = x.reshape(b, c, T).copy()
    for bi in range(b):
        out[bi] += parts[2 * bi] + parts[2 * bi + 1]
    out += proj_b.reshape(1, c, 1)
    return out.reshape(b, c, hh, ww).astype(np.float32)
